# revision 1
# baseline (speedup 1.0000x reference)
"""Divergence-free RBF kernel Gram matrix on 8 Trainium2 NeuronCores.

Math: for d=2, with scaled coords x' = x*exp(-ll/2):
  dx = x0_i - y0_j, dy = x1_i - y1_j, r2 = dx^2 + dy^2, e = exp(-r2/2)
  K[2i+0, 2j+0] = e * (1 - dy^2)
  K[2i+0, 2j+1] = K[2i+1, 2j+0] = e * dx*dy
  K[2i+1, 2j+1] = e * (1 - dx^2)

Each polynomial factor is low-rank in the basis {1, x0, x1, x0*x1, x0^2, x1^2}
(K=6): host precomputes L [6, n] (X side) and column-interleaved R [6, 2m]
(Y side), device builds the polynomial matrices with PE matmuls, exp on ACT,
and one DVE multiply per output element. fp32-grade matmul precision comes
from a hi/lo bf16 split stacked to K=18: [Lhi;Llo;Lhi].T @ [Rhi;Rhi;Rlo].

Sharding: rows of X (n axis) split across 8 cores, 512 each -> each core
writes 1024 output rows of the (8192, 8192) Gram matrix. No communication.
"""

import numpy as np
import ml_dtypes

N = 4096          # X rows
M = 4096          # Y rows
D = 2
NCORES = 8
NPC = N // NCORES  # 512 X rows per core
IB = 128           # i-block = partition count
NIB = NPC // IB    # 4 i-blocks per core
JG = 256           # j-group size (j count per PSUM tile)
NJG = M // JG      # 16 j-groups
KST = 18           # stacked contraction dim (3 x 6 basis rows)

_cache = {}


def _hi_lo(a):
    bf = ml_dtypes.bfloat16
    hi = a.astype(bf)
    lo = (a - hi.astype(np.float64)).astype(bf)
    return hi, lo


def _prepare_inputs(X, Y, log_length_scale):
    s = float(np.exp(-0.5 * np.float64(np.asarray(log_length_scale).reshape(-1)[0])))
    xs = np.asarray(X, dtype=np.float64).reshape(N, D) * s
    ys = np.asarray(Y, dtype=np.float64).reshape(M, D) * s
    x0, x1 = xs[:, 0], xs[:, 1]
    y0, y1 = ys[:, 0], ys[:, 1]
    one_n, zero_m, one_m = np.ones(N), np.zeros(M), np.ones(M)

    # X-side basis [6, N]: rows {1, x0, x1, x0*x1, x0^2, x1^2}
    L = np.stack([one_n, x0, x1, x0 * x1, x0 ** 2, x1 ** 2])

    # Y-side coefficient columns [6, M] per output channel
    c_dxdy = np.stack([y0 * y1, -y1, -y0, one_m, zero_m, zero_m])
    c_00 = np.stack([1 - y1 ** 2, zero_m, 2 * y1, zero_m, zero_m, -one_m])
    c_11 = np.stack([1 - y0 ** 2, 2 * y0, zero_m, zero_m, -one_m, zero_m])
    c_r2 = np.stack([y0 ** 2 + y1 ** 2, -2 * y0, -2 * y1, zero_m, one_m, one_m])

    Re = np.zeros((6, 2 * M))   # even output rows: [1-dy^2 | dxdy] interleaved
    Re[:, 0::2] = c_00
    Re[:, 1::2] = c_dxdy
    Ro = np.zeros((6, 2 * M))   # odd output rows: [dxdy | 1-dx^2] interleaved
    Ro[:, 0::2] = c_dxdy
    Ro[:, 1::2] = c_11

    # Merge Re/Ro into one tensor so each j-group is a single N=1024 matmul:
    # group g occupies cols [1024g, 1024g+1024) = [Re_g (512) | Ro_g (512)]
    Reo = np.zeros((6, 4 * M))
    v = Reo.reshape(6, 2 * M // 512, 2, 512)
    v[:, :, 0, :] = Re.reshape(6, -1, 512)
    v[:, :, 1, :] = Ro.reshape(6, -1, 512)

    Lh, Ll = _hi_lo(L)
    Lst = np.ascontiguousarray(np.concatenate([Lh, Ll, Lh], axis=0))  # (18, N)

    def r_stack(R):
        Rh, Rl = _hi_lo(R)
        return np.ascontiguousarray(np.concatenate([Rh, Rh, Rl], axis=0))

    return Lst, r_stack(Reo), r_stack(c_r2)


def _build_module(bass_cls=None, reps=1, **bass_kw):
    from concourse import bacc, mybir
    import concourse.tile as tile

    bf16 = mybir.dt.bfloat16
    f32 = mybir.dt.float32
    Exp = mybir.ActivationFunctionType.Exp

    if bass_cls is None:
        bass_cls = bacc.Bacc
    nc = bass_cls("TRN2", target_bir_lowering=False, debug=False,
                  enable_asserts=False, **bass_kw)
    lhsT_d = nc.dram_tensor("lhsT", [KST, NPC], bf16, kind="ExternalInput")
    reo_d = nc.dram_tensor("r_eo", [KST, 4 * M], bf16, kind="ExternalInput")
    rr_d = nc.dram_tensor("r_r2", [KST, M], bf16, kind="ExternalInput")
    out_d = nc.dram_tensor("out", [2 * NPC, 2 * M], f32, kind="ExternalOutput")

    QJ = 4 * JG  # 1024 j's covered by one r2/exp quad

    with tile.TileContext(nc) as tc:
        with (
            tc.tile_pool(name="const", bufs=1) as cpool,
            tc.tile_pool(name="outp", bufs=2) as opool,
            tc.tile_pool(name="ep", bufs=3) as epool,
            tc.tile_pool(name="ps", bufs=2, space="PSUM") as ppool,
        ):
            lhsT = cpool.tile([KST, NPC], bf16)
            nc.sync.dma_start(out=lhsT[:], in_=lhsT_d[:, :])
            reo_sb = cpool.tile([KST, 4 * M], bf16)
            nc.sync.dma_start(out=reo_sb[:], in_=reo_d[:, :])
            rr_sb = cpool.tile([KST, M], bf16)
            nc.sync.dma_start(out=rr_sb[:], in_=rr_d[:, :])

            out_view = out_d.ap().rearrange("(i t) c -> i t c", t=2)

            for ib in [i for _ in range(reps) for i in range(NIB)]:
                wt = lhsT[:, ib * IB:(ib + 1) * IB]
                # halves: [0:8192) even output rows, [8192:16384) odd rows
                out_all = opool.tile([IB, 4 * M], f32, tag="out_all")
                out_q = out_all[:].rearrange("p (h j t) -> p h j t", h=2, t=2)
                for q in range(M // QJ):
                    r2q = ppool.tile([IB, QJ], f32, tag="r2")
                    for s in range(QJ // 512):
                        nc.tensor.matmul(
                            r2q[:, s * 512:(s + 1) * 512], wt,
                            rr_sb[:, q * QJ + s * 512:q * QJ + (s + 1) * 512],
                            start=True, stop=True)
                    ebig = epool.tile([IB, QJ], f32, tag="e")
                    nc.scalar.activation(ebig[:], r2q[:], Exp, scale=-0.5)
                    for h in range(QJ // JG):
                        g = q * (QJ // JG) + h
                        memo = ppool.tile([IB, 4 * JG], f32, tag="memo")
                        for s in range(4 * JG // 512):
                            nc.tensor.matmul(
                                memo[:, s * 512:(s + 1) * 512], wt,
                                reo_sb[:, g * 4 * JG + s * 512:
                                       g * 4 * JG + (s + 1) * 512],
                                start=True, stop=True)
                        eb = (ebig[:, h * JG:(h + 1) * JG]
                              .unsqueeze(1).unsqueeze(3)
                              .broadcast_to([IB, 2, JG, 2]))
                        nc.vector.tensor_mul(
                            out_q[:, :, g * JG:(g + 1) * JG, :],
                            memo[:].rearrange("p (h j t) -> p h j t", h=2, t=2),
                            eb,
                        )
                i0 = ib * IB
                nc.sync.dma_start(out=out_view[i0:i0 + IB, 0:1, :].squeeze(1),
                                  in_=out_all[:, 0:2 * M])
                nc.sync.dma_start(out=out_view[i0:i0 + IB, 1:2, :].squeeze(1),
                                  in_=out_all[:, 2 * M:4 * M])
    nc.finalize()
    return nc


def _run(X, Y, log_length_scale, trace=False):
    from concourse.bass_utils import run_bass_kernel_spmd

    Lst, Reo, Rr = _prepare_inputs(X, Y, log_length_scale)
    if "nc" not in _cache:
        _cache["nc"] = _build_module()
    nc = _cache["nc"]
    in_maps = [
        {
            "lhsT": np.ascontiguousarray(Lst[:, c * NPC:(c + 1) * NPC]),
            "r_eo": Reo,
            "r_r2": Rr,
        }
        for c in range(NCORES)
    ]
    res = run_bass_kernel_spmd(nc, in_maps, core_ids=list(range(NCORES)),
                               trace=trace)
    out = np.concatenate([r["out"] for r in res.results], axis=0)
    return out.reshape(1, 2 * N, 2 * M), res


def kernel(X, Y, log_length_scale):
    out, _ = _run(np.asarray(X), np.asarray(Y), np.asarray(log_length_scale))
    return out



# revision 2
# speedup vs baseline: 1.1134x; 1.1134x over previous
"""Divergence-free RBF kernel Gram matrix on 8 Trainium2 NeuronCores.

Math: for d=2, with scaled coords x' = x*exp(-ll/2):
  dx = x0_i - y0_j, dy = x1_i - y1_j, r2 = dx^2 + dy^2, e = exp(-r2/2)
  K[2i+0, 2j+0] = e * (1 - dy^2)          (channel A)
  K[2i+0, 2j+1] = K[2i+1, 2j+0] = e*dx*dy (channel X)
  K[2i+1, 2j+1] = e * (1 - dx^2)          (channel C)

Each polynomial factor is low-rank in the basis {1, x0, x1, x0*x1, x0^2, x1^2}
(6 terms): host precomputes X-side rows L and Y-side coefficient columns, the
device builds A/X/C/r2 with PE matmuls, exp on ACT, and one multiply-by-e per
output element.

Precision/perf scheme: fp8(e4m3) hi/lo 4-term split (24 contraction rows as
12 partitions x 2 k-tiles) with DoubleRow perf mode -> 2 matmul cols/cycle.
Output is written bf16 (rel-err budget 2e-2; this lands ~4e-3) which halves
the HBM write traffic - the dominant cost in this memory-bound problem.

Per 512-j group the PSUM tile is laid out [A,X interleaved | C | r2] so every
consumer read is contiguous: ACT does exp(r2), DVE multiplies the (A,X) pair
block by e (broadcast), the C-plane multiply alternates between DVE and
(ACT copy + GpSimd multiply) since GpSimd cannot read PSUM, and the duplicate
X column of the odd output row is a cheap SBUF copy.

Sharding: rows of X (n axis) split across 8 cores, 512 each -> each core
writes 1024 rows of the (8192, 8192) Gram matrix. No communication.
"""

import numpy as np
import ml_dtypes

N = 4096          # X rows
M = 4096          # Y rows
NCORES = 8
NPC = N // NCORES  # 512 X rows per core
IB = 128           # i-block = partition count
NIB = NPC // IB    # 4 i-blocks per core
JG = 512           # j's per jgroup
NJG = M // JG      # 8 jgroups (per i-block)
KT = 2             # fp8 DoubleRow k-tiles
KP = 12            # contraction partitions (24 rows = 12 x 2)

# Engine routing per global jgroup index (ib*NJG + g), tuned for balance:
# C-plane multiply on DVE every 4th group, else ACT-copy + GpSimd-multiply.
# X-duplicate copy rides on GpSimd when DVE took the C multiply, else ACT.
def _c_on_dve(gg):
    return gg % 4 == 0

_cache = {}


def _hi_lo(a, dt):
    hi = a.astype(dt)
    lo = (a - hi.astype(np.float64)).astype(dt)
    return hi, lo


def _prepare_inputs(X, Y, log_length_scale):
    f8 = ml_dtypes.float8_e4m3
    s = float(np.exp(-0.5 * np.float64(np.asarray(log_length_scale).reshape(-1)[0])))
    xs = np.asarray(X, dtype=np.float64).reshape(N, 2) * s
    ys = np.asarray(Y, dtype=np.float64).reshape(M, 2) * s
    x0, x1 = xs[:, 0], xs[:, 1]
    y0, y1 = ys[:, 0], ys[:, 1]
    one_n, zero_m, one_m = np.ones(N), np.zeros(M), np.ones(M)

    # X-side basis rows {1, x0, x1, x0*x1, x0^2, x1^2}, hi/lo 4-term stack
    L = np.stack([one_n, x0, x1, x0 * x1, x0 ** 2, x1 ** 2])
    Lh, Ll = _hi_lo(L, f8)
    L24 = np.concatenate([Lh, Ll, Lh, Ll], axis=0)  # pairs with [Rh;Rh;Rl;Rl]

    c_00 = np.stack([1 - y1 ** 2, zero_m, 2 * y1, zero_m, zero_m, -one_m])
    c_dxdy = np.stack([y0 * y1, -y1, -y0, one_m, zero_m, zero_m])
    c_11 = np.stack([1 - y0 ** 2, 2 * y0, zero_m, zero_m, -one_m, zero_m])
    c_r2 = np.stack([y0 ** 2 + y1 ** 2, -2 * y0, -2 * y1, zero_m, one_m, one_m])

    def r24(c):
        ch, cl = _hi_lo(c, f8)
        return np.concatenate([ch, ch, cl, cl], axis=0).astype(np.float64)

    R_A, R_X, R_C, R_R = r24(c_00), r24(c_dxdy), r24(c_11), r24(c_r2)

    # Stationary: [128, KT, N]; band b (partitions 32b..32b+11) replicates
    # the 24 rows as 2 k-tile planes.
    wts = np.zeros((128, KT, N), f8)
    for b in range(4):
        wts[32 * b:32 * b + 12, 0, :] = L24[0:12]
        wts[32 * b:32 * b + 12, 1, :] = L24[12:24]

    # Moving tensor: per jgroup 2048 cols = [A,X pairs (1024) | C (512) | R
    # (512)]; band b holds jgroups {2b, 2b+1} as the two hh-halves; k-tile
    # innermost.
    jidx = np.arange(M).reshape(NJG, JG)
    blk = np.zeros((24, NJG, 2048))
    blk[:, :, 0:1024:2] = R_A[:, jidx]
    blk[:, :, 1:1024:2] = R_X[:, jidx]
    blk[:, :, 1024:1536] = R_C[:, jidx]
    blk[:, :, 1536:2048] = R_R[:, jidx]
    rhs = np.zeros((128, 2, 2048, KT), f8)
    blk8 = blk.astype(f8)
    for b in range(4):
        rhs[32 * b:32 * b + 12, :, :, 0] = blk8[0:12, 2 * b:2 * b + 2, :]
        rhs[32 * b:32 * b + 12, :, :, 1] = blk8[12:24, 2 * b:2 * b + 2, :]

    return wts, np.ascontiguousarray(rhs.reshape(128, 8192))


def _build_module(bass_cls=None, **bass_kw):
    from concourse import bacc, mybir
    import concourse.tile as tile

    f8 = mybir.dt.float8e4
    bf16 = mybir.dt.bfloat16
    f32 = mybir.dt.float32
    Exp = mybir.ActivationFunctionType.Exp
    DR = mybir.MatmulPerfMode.DoubleRow

    if bass_cls is None:
        bass_cls = bacc.Bacc
    nc = bass_cls("TRN2", target_bir_lowering=False, debug=False,
                  enable_asserts=False, **bass_kw)
    wts_d = nc.dram_tensor("wts", [128, KT * NPC], f8, kind="ExternalInput")
    rhs_d = nc.dram_tensor("rhs", [128, 8192], f8, kind="ExternalInput")
    out_d = nc.dram_tensor("out", [2 * NPC, 2 * M], bf16, kind="ExternalOutput")

    with tile.TileContext(nc) as tc:
        with (
            tc.tile_pool(name="const", bufs=1) as cpool,
            tc.tile_pool(name="outp", bufs=2) as opool,
            tc.tile_pool(name="ep", bufs=3) as epool,
            tc.tile_pool(name="scp", bufs=3) as scpool,
            tc.tile_pool(name="ps", bufs=2, space="PSUM") as ppool,
        ):
            wts_sb = cpool.tile([128, KT * NPC], f8)
            nc.sync.dma_start(out=wts_sb[:], in_=wts_d[:, :])
            rhs_sb = cpool.tile([128, 8192], f8)
            # chunked so jgroup 0's data (first half) lands first
            nc.sync.dma_start(out=rhs_sb[:, 0:4096], in_=rhs_d[:, 0:4096])
            nc.sync.dma_start(out=rhs_sb[:, 4096:8192], in_=rhs_d[:, 4096:8192])

            out_view = out_d.ap().rearrange("(i t) c -> i t c", t=2)

            for ib in range(NIB):
                out_all = opool.tile([IB, 4 * M], bf16, tag="out_all")
                for g in range(NJG):
                    gg = ib * NJG + g
                    b, hh = g // 2, g % 2
                    band = wts_sb[32 * b:32 * b + 12, :]
                    wt = band.rearrange("p (k m) -> p k m", k=KT)[
                        :, :, ib * IB:(ib + 1) * IB]
                    rband = rhs_sb[32 * b:32 * b + 12, :].rearrange(
                        "p (h c k) -> p h c k", h=2, k=KT)
                    memo = ppool.tile([IB, 2048], f32, tag="memo")
                    for s in range(4):
                        rh = rband[:, hh, s * 512:(s + 1) * 512, :]
                        rh = rh.transpose([0, 2, 1])  # [12, kt, 512]
                        nc.tensor.matmul(
                            memo[:, s * 512:(s + 1) * 512], wt, rh,
                            start=True, stop=True, perf_mode=DR,
                            tile_position=(32 * b, 0))

                    ebig = epool.tile([IB, JG], f32, tag="e")
                    nc.scalar.activation(ebig[:], memo[:, 1536:2048], Exp,
                                         scale=-0.5)

                    # h0 = (A,X) pairs * e
                    out0 = out_all[:, g * 1024:(g + 1) * 1024].rearrange(
                        "p (j t) -> p j t", t=2)
                    in0 = memo[:, 0:1024].rearrange("p (j t) -> p j t", t=2)
                    eb = ebig[:].unsqueeze(2).broadcast_to([IB, JG, 2])
                    nc.vector.tensor_mul(out0, in0, eb)

                    # h1 strided views (t=0 even col, t=1 odd col)
                    h1 = out_all[:, 8192 + g * 1024:8192 + (g + 1) * 1024]
                    h1 = h1.rearrange("p (j t) -> p j t", t=2)
                    h1e = h1[:, :, 0:1].squeeze(2)
                    h1o = h1[:, :, 1:2].squeeze(2)
                    h0o = out0[:, :, 1:2].squeeze(2)

                    if _c_on_dve(gg):
                        nc.vector.tensor_mul(h1o, memo[:, 1024:1536], ebig[:])
                        nc.gpsimd.tensor_scalar_mul(h1e, h0o, 1.0)
                    else:
                        sc = scpool.tile([IB, JG], bf16, tag="sc")
                        nc.scalar.copy(sc[:], memo[:, 1024:1536])
                        nc.gpsimd.tensor_mul(h1o, sc[:], ebig[:])
                        nc.scalar.copy(h1e, h0o)

                    if hh == 1:
                        i0 = ib * IB
                        cb = (g - 1) * 1024
                        nc.sync.dma_start(
                            out=out_view[i0:i0 + IB, 0:1, cb:cb + 2048].squeeze(1),
                            in_=out_all[:, cb:cb + 2048])
                        nc.sync.dma_start(
                            out=out_view[i0:i0 + IB, 1:2, cb:cb + 2048].squeeze(1),
                            in_=out_all[:, 8192 + cb:8192 + cb + 2048])
    nc.finalize()
    return nc


def _run(X, Y, log_length_scale, trace=False):
    from concourse.bass_utils import run_bass_kernel_spmd

    wts, rhs = _prepare_inputs(X, Y, log_length_scale)
    if "nc" not in _cache:
        _cache["nc"] = _build_module()
    nc = _cache["nc"]
    in_maps = [
        {
            "wts": np.ascontiguousarray(
                wts[:, :, c * NPC:(c + 1) * NPC].reshape(128, KT * NPC)),
            "rhs": rhs,
        }
        for c in range(NCORES)
    ]
    res = run_bass_kernel_spmd(nc, in_maps, core_ids=list(range(NCORES)),
                               trace=trace)
    out = np.concatenate([r["out"].astype(np.float32) for r in res.results],
                         axis=0)
    return out.reshape(1, 2 * N, 2 * M), res


def kernel(X, Y, log_length_scale):
    out, _ = _run(np.asarray(X), np.asarray(Y), np.asarray(log_length_scale))
    return out


# revision 4
# speedup vs baseline: 1.4376x; 1.2912x over previous
"""Divergence-free RBF kernel Gram matrix on 8 Trainium2 NeuronCores.

Math: for d=2, with scaled coords x' = x*exp(-ll/2):
  dx = x0_i - y0_j, dy = x1_i - y1_j, r2 = dx^2 + dy^2, e = exp(-r2/2)
  K[2i+0, 2j+0] = e * (1 - dy^2)          (channel A)
  K[2i+0, 2j+1] = K[2i+1, 2j+0] = e*dx*dy (channel X)
  K[2i+1, 2j+1] = e * (1 - dx^2)          (channel C)

Each polynomial factor is low-rank in the basis {1, x0, x1, x0*x1, x0^2, x1^2}
(6 terms): host precomputes X-side rows L and Y-side coefficient columns, the
device builds A/X/C/r2 with PE matmuls, exp on ACT, and one multiply-by-e per
output element.

Precision/perf scheme: fp8(e4m3) hi/lo 4-term split (24 contraction rows as
12 partitions x 2 k-tiles) with DoubleRow perf mode -> 2 matmul cols/cycle.
Output is written bf16 (rel-err budget 2e-2; this lands ~4e-3) which halves
the HBM write traffic - the dominant cost in this memory-bound problem.

Per 512-j group the PSUM tile is laid out [A,X interleaved | C | r2] so every
consumer read is contiguous: ACT does exp(r2), DVE multiplies the (A,X) pair
block by e (broadcast), the C-plane multiply alternates between DVE and
(ACT copy + GpSimd multiply) since GpSimd cannot read PSUM, and the duplicate
X column of the odd output row is a cheap SBUF copy.

Sharding: rows of X (n axis) split across 8 cores, 512 each -> each core
writes 1024 rows of the (8192, 8192) Gram matrix. No communication.
"""

import numpy as np
import ml_dtypes

N = 4096          # X rows
M = 4096          # Y rows
NCORES = 8
NPC = N // NCORES  # 512 X rows per core
IB = 128           # i-block = partition count
NIB = NPC // IB    # 4 i-blocks per core
JG = 512           # j's per jgroup
NJG = M // JG      # 8 jgroups (per i-block)
KT = 2             # fp8 DoubleRow k-tiles
KP = 12            # contraction partitions (24 rows = 12 x 2)

# Engine routing per global jgroup index (ib*NJG + g), tuned for balance:
# C-plane multiply mostly on DVE, else ACT-copy + GpSimd-multiply. GpSimd
# never touches strided 2-byte patterns (catastrophic on Q7) or PSUM.
def _c_on_dve(gg):
    return gg % 8 < 5

_cache = {}


def _hi_lo(a, dt):
    hi = a.astype(dt)
    lo = (a - hi.astype(np.float64)).astype(dt)
    return hi, lo


def _prepare_inputs(X, Y, log_length_scale):
    f8 = ml_dtypes.float8_e4m3
    s = float(np.exp(-0.5 * np.float64(np.asarray(log_length_scale).reshape(-1)[0])))
    xs = np.asarray(X, dtype=np.float64).reshape(N, 2) * s
    ys = np.asarray(Y, dtype=np.float64).reshape(M, 2) * s
    x0, x1 = xs[:, 0], xs[:, 1]
    y0, y1 = ys[:, 0], ys[:, 1]
    one_n, zero_m, one_m = np.ones(N), np.zeros(M), np.ones(M)

    # X-side basis rows {1, x0, x1, x0*x1, x0^2, x1^2}, hi/lo 4-term stack
    L = np.stack([one_n, x0, x1, x0 * x1, x0 ** 2, x1 ** 2])
    Lh, Ll = _hi_lo(L, f8)
    L24 = np.concatenate([Lh, Ll, Lh, Ll], axis=0)  # pairs with [Rh;Rh;Rl;Rl]

    c_00 = np.stack([1 - y1 ** 2, zero_m, 2 * y1, zero_m, zero_m, -one_m])
    c_dxdy = np.stack([y0 * y1, -y1, -y0, one_m, zero_m, zero_m])
    c_11 = np.stack([1 - y0 ** 2, 2 * y0, zero_m, zero_m, -one_m, zero_m])
    c_r2 = np.stack([y0 ** 2 + y1 ** 2, -2 * y0, -2 * y1, zero_m, one_m, one_m])

    def r24(c):
        ch, cl = _hi_lo(c, f8)
        return np.concatenate([ch, ch, cl, cl], axis=0).astype(np.float64)

    R_A, R_X, R_C, R_R = r24(c_00), r24(c_dxdy), r24(c_11), r24(c_r2)

    # Stationary: [128, KT, N]; band b (partitions 32b..32b+11) replicates
    # the 24 rows as 2 k-tile planes.
    wts = np.zeros((128, KT, N), f8)
    for b in range(4):
        wts[32 * b:32 * b + 12, 0, :] = L24[0:12]
        wts[32 * b:32 * b + 12, 1, :] = L24[12:24]

    # Moving tensor: per jgroup 2048 cols = [A,X pairs (1024) | C (512) | R
    # (512)]; band b holds jgroups {2b, 2b+1} as the two hh-halves; k-tile
    # innermost.
    jidx = np.arange(M).reshape(NJG, JG)
    blk = np.zeros((24, NJG, 2048))
    blk[:, :, 0:1024:2] = R_A[:, jidx]
    blk[:, :, 1:1024:2] = R_X[:, jidx]
    blk[:, :, 1024:1536] = R_C[:, jidx]
    blk[:, :, 1536:2048] = R_R[:, jidx]
    rhs = np.zeros((128, 2, 2048, KT), f8)
    blk8 = blk.astype(f8)
    for b in range(4):
        rhs[32 * b:32 * b + 12, :, :, 0] = blk8[0:12, 2 * b:2 * b + 2, :]
        rhs[32 * b:32 * b + 12, :, :, 1] = blk8[12:24, 2 * b:2 * b + 2, :]

    return wts, np.ascontiguousarray(rhs.reshape(128, 8192))


def _build_module(bass_cls=None, **bass_kw):
    from concourse import bacc, mybir
    import concourse.tile as tile

    f8 = mybir.dt.float8e4
    bf16 = mybir.dt.bfloat16
    f32 = mybir.dt.float32
    Exp = mybir.ActivationFunctionType.Exp
    DR = mybir.MatmulPerfMode.DoubleRow

    if bass_cls is None:
        bass_cls = bacc.Bacc
    nc = bass_cls("TRN2", target_bir_lowering=False, debug=False,
                  enable_asserts=False, **bass_kw)
    wts_d = nc.dram_tensor("wts", [128, KT * NPC], f8, kind="ExternalInput")
    rhs_d = nc.dram_tensor("rhs", [128, 8192], f8, kind="ExternalInput")
    out_d = nc.dram_tensor("out", [2 * NPC, 2 * M], bf16, kind="ExternalOutput")

    with tile.TileContext(nc) as tc:
        with (
            tc.tile_pool(name="const", bufs=1) as cpool,
            tc.tile_pool(name="outp", bufs=2) as opool,
            tc.tile_pool(name="ep", bufs=3) as epool,
            tc.tile_pool(name="scp", bufs=3) as scpool,
            tc.tile_pool(name="ps", bufs=2, space="PSUM") as ppool,
        ):
            wts_sb = cpool.tile([128, KT * NPC], f8)
            nc.sync.dma_start(out=wts_sb[:], in_=wts_d[:, :])
            rhs_sb = cpool.tile([128, 8192], f8)
            # chunked so jgroup 0's data (first half) lands first
            nc.sync.dma_start(out=rhs_sb[:, 0:4096], in_=rhs_d[:, 0:4096])
            nc.sync.dma_start(out=rhs_sb[:, 4096:8192], in_=rhs_d[:, 4096:8192])

            out_view = out_d.ap().rearrange("(i t) c -> i t c", t=2)

            for ib in range(NIB):
                out_all = opool.tile([IB, 4 * M], bf16, tag="out_all")
                for g in range(NJG):
                    gg = ib * NJG + g
                    b, hh = g // 2, g % 2
                    band = wts_sb[32 * b:32 * b + 12, :]
                    wt = band.rearrange("p (k m) -> p k m", k=KT)[
                        :, :, ib * IB:(ib + 1) * IB]
                    rband = rhs_sb[32 * b:32 * b + 12, :].rearrange(
                        "p (h c k) -> p h c k", h=2, k=KT)
                    # Split PSUM tiles: WAR deps of future matmuls bind only
                    # to the consumers of the matching half. CR first so the
                    # longest consumer chain (exp -> C route) starts early.
                    mcr = ppool.tile([IB, 1024], f32, tag="memo_cr")
                    max_ = ppool.tile([IB, 1024], f32, tag="memo_ax")

                    def mm(out, s):
                        rh = rband[:, hh, s * 512:(s + 1) * 512, :]
                        rh = rh.transpose([0, 2, 1])  # [12, kt, 512]
                        nc.tensor.matmul(
                            out, wt, rh, start=True, stop=True, perf_mode=DR,
                            tile_position=(32 * b, 0))

                    mm(mcr[:, 512:1024], 3)   # r2 plane
                    mm(mcr[:, 0:512], 2)      # C plane
                    ebig = epool.tile([IB, JG], f32, tag="e")
                    nc.scalar.activation(ebig[:], mcr[:, 512:1024], Exp,
                                         scale=-0.5)
                    mm(max_[:, 0:512], 0)     # (A,X) pairs
                    mm(max_[:, 512:1024], 1)

                    # h0 = (A,X) pairs * e
                    out0 = out_all[:, g * 1024:(g + 1) * 1024].rearrange(
                        "p (j t) -> p j t", t=2)
                    in0 = max_[:].rearrange("p (j t) -> p j t", t=2)
                    eb = ebig[:].unsqueeze(2).broadcast_to([IB, JG, 2])
                    nc.vector.tensor_mul(out0, in0, eb)

                    # h1 strided views (t=0 even col, t=1 odd col)
                    h1 = out_all[:, 8192 + g * 1024:8192 + (g + 1) * 1024]
                    h1 = h1.rearrange("p (j t) -> p j t", t=2)
                    h1e = h1[:, :, 0:1].squeeze(2)
                    h1o = h1[:, :, 1:2].squeeze(2)
                    h0o = out0[:, :, 1:2].squeeze(2)

                    if _c_on_dve(gg):
                        nc.vector.tensor_mul(h1o, mcr[:, 0:512], ebig[:])
                    else:
                        sc = scpool.tile([IB, JG], bf16, tag="sc")
                        nc.scalar.copy(sc[:], mcr[:, 0:512])
                        nc.gpsimd.tensor_mul(h1o, sc[:], ebig[:])
                    nc.scalar.copy(h1e, h0o)

                    if hh == 1:
                        i0 = ib * IB
                        cb = (g - 1) * 1024
                        nc.sync.dma_start(
                            out=out_view[i0:i0 + IB, 0:1, cb:cb + 2048].squeeze(1),
                            in_=out_all[:, cb:cb + 2048])
                        nc.sync.dma_start(
                            out=out_view[i0:i0 + IB, 1:2, cb:cb + 2048].squeeze(1),
                            in_=out_all[:, 8192 + cb:8192 + cb + 2048])
    nc.finalize()
    return nc


def _run(X, Y, log_length_scale, trace=False):
    from concourse.bass_utils import run_bass_kernel_spmd

    wts, rhs = _prepare_inputs(X, Y, log_length_scale)
    if "nc" not in _cache:
        _cache["nc"] = _build_module()
    nc = _cache["nc"]
    in_maps = [
        {
            "wts": np.ascontiguousarray(
                wts[:, :, c * NPC:(c + 1) * NPC].reshape(128, KT * NPC)),
            "rhs": rhs,
        }
        for c in range(NCORES)
    ]
    res = run_bass_kernel_spmd(nc, in_maps, core_ids=list(range(NCORES)),
                               trace=trace)
    out = np.concatenate([r["out"].astype(np.float32) for r in res.results],
                         axis=0)
    return out.reshape(1, 2 * N, 2 * M), res


def kernel(X, Y, log_length_scale):
    out, _ = _run(np.asarray(X), np.asarray(Y), np.asarray(log_length_scale))
    return out


# revision 12
# speedup vs baseline: 1.4776x; 1.0278x over previous
"""Divergence-free RBF kernel Gram matrix on 8 Trainium2 NeuronCores.

Math: for d=2, with scaled coords x' = x*exp(-ll/2):
  dx = x0_i - y0_j, dy = x1_i - y1_j, r2 = dx^2 + dy^2, e = exp(-r2/2)
  K[2i+0, 2j+0] = e * (1 - dy^2)          (channel A)
  K[2i+0, 2j+1] = K[2i+1, 2j+0] = e*dx*dy (channel X)
  K[2i+1, 2j+1] = e * (1 - dx^2)          (channel C)

Each polynomial factor is low-rank in the basis {1, x0, x1, x0*x1, x0^2, x1^2}
(6 terms): host precomputes X-side rows L and Y-side coefficient columns, the
device builds A/X/C/r2 with PE matmuls, exp on ACT, and one multiply-by-e per
output element.

Precision/perf scheme: fp8(e4m3) hi/lo 4-term split (24 contraction rows as
12 partitions x 2 k-tiles) with DoubleRow perf mode -> 2 matmul cols/cycle.
Output is written bf16 (rel-err budget 2e-2; this lands ~4e-3) which halves
the HBM write traffic - the dominant cost in this memory-bound problem.

Per 512-j group the PSUM tile is laid out [A,X interleaved | C | r2] so every
consumer read is contiguous: ACT does exp(r2), DVE multiplies the (A,X) pair
block by e (broadcast), the C-plane multiply alternates between DVE and
(ACT copy + GpSimd multiply) since GpSimd cannot read PSUM, and the duplicate
X column of the odd output row is a cheap SBUF copy.

Sharding: rows of X (n axis) split across 8 cores, 512 each -> each core
writes 1024 rows of the (8192, 8192) Gram matrix. No communication.
"""

import numpy as np
import ml_dtypes

N = 4096          # X rows
M = 4096          # Y rows
NCORES = 8
NPC = N // NCORES  # 512 X rows per core
IB = 128           # i-block = partition count
NIB = NPC // IB    # 4 i-blocks per core
JG = 512           # j's per jgroup
NJG = M // JG      # 8 jgroups (per i-block)
KT = 2             # fp8 DoubleRow k-tiles
KP = 12            # contraction partitions (24 rows = 12 x 2)

# Engine routing per global jgroup index (ib*NJG + g), tuned for balance:
# C-plane multiply mostly on DVE, else ACT-copy + GpSimd-multiply. GpSimd
# never touches strided 2-byte patterns (catastrophic on Q7) or PSUM.
def _c_on_dve(gg):
    return gg % 8 < 5 or gg >= 29

_cache = {}


def _hi_lo(a, dt):
    hi = a.astype(dt)
    lo = (a - hi.astype(np.float64)).astype(dt)
    return hi, lo


def _prepare_inputs(X, Y, log_length_scale):
    f8 = ml_dtypes.float8_e4m3
    s = float(np.exp(-0.5 * np.float64(np.asarray(log_length_scale).reshape(-1)[0])))
    xs = np.asarray(X, dtype=np.float64).reshape(N, 2) * s
    ys = np.asarray(Y, dtype=np.float64).reshape(M, 2) * s
    x0, x1 = xs[:, 0], xs[:, 1]
    y0, y1 = ys[:, 0], ys[:, 1]
    one_n, zero_m, one_m = np.ones(N), np.zeros(M), np.ones(M)

    # X-side basis rows {1, x0, x1, x0*x1, x0^2, x1^2}, hi/lo 4-term stack
    L = np.stack([one_n, x0, x1, x0 * x1, x0 ** 2, x1 ** 2])
    Lh, Ll = _hi_lo(L, f8)
    L24 = np.concatenate([Lh, Ll, Lh, Ll], axis=0)  # pairs with [Rh;Rh;Rl;Rl]

    c_00 = np.stack([1 - y1 ** 2, zero_m, 2 * y1, zero_m, zero_m, -one_m])
    c_dxdy = np.stack([y0 * y1, -y1, -y0, one_m, zero_m, zero_m])
    c_11 = np.stack([1 - y0 ** 2, 2 * y0, zero_m, zero_m, -one_m, zero_m])
    c_r2 = np.stack([y0 ** 2 + y1 ** 2, -2 * y0, -2 * y1, zero_m, one_m, one_m])

    def r24(c):
        ch, cl = _hi_lo(c, f8)
        return np.concatenate([ch, ch, cl, cl], axis=0).astype(np.float64)

    R_A, R_X, R_C, R_R = r24(c_00), r24(c_dxdy), r24(c_11), r24(c_r2)

    # Stationary: [128, KT, N]; band b (partitions 32b..32b+11) replicates
    # the 24 rows as 2 k-tile planes.
    wts = np.zeros((128, KT, N), f8)
    for b in range(4):
        wts[32 * b:32 * b + 12, 0, :] = L24[0:12]
        wts[32 * b:32 * b + 12, 1, :] = L24[12:24]

    # Moving tensor: per jgroup 2048 cols, k-tile PLANAR (DoubleRow wants
    # planar k-subtiles), CR block first so exp's inputs stream in first.
    # Per (band, hh-half) free layout (4096 fp8):
    #   [C k0 (512) | R k0 (512) | C k1 | R k1 | AX k0 (1024) | AX k1 (1024)]
    jidx = np.arange(M).reshape(NJG, JG)
    blkC = R_C[:, jidx].astype(f8)          # [24, NJG, 512]
    blkR = R_R[:, jidx].astype(f8)
    blkAX = np.zeros((24, NJG, 1024))
    blkAX[:, :, 0::2] = R_A[:, jidx]
    blkAX[:, :, 1::2] = R_X[:, jidx]
    blkAX = blkAX.astype(f8)
    rhs = np.zeros((128, 2, 4096), f8)
    for b in range(4):
        rows = slice(32 * b, 32 * b + 12)
        halves = slice(2 * b, 2 * b + 2)
        for kt in range(KT):
            r24 = slice(12 * kt, 12 * kt + 12)
            rhs[rows, :, kt * 1024 + 0:kt * 1024 + 512] = blkC[r24, halves]
            rhs[rows, :, kt * 1024 + 512:kt * 1024 + 1024] = blkR[r24, halves]
            rhs[rows, :, 2048 + kt * 1024:2048 + (kt + 1) * 1024] = \
                blkAX[r24, halves]

    return wts, np.ascontiguousarray(rhs.reshape(128, 8192))


def _build_module(bass_cls=None, **bass_kw):
    from concourse import bacc, mybir
    import concourse.tile as tile
    from concourse.ap import AP as BAP

    f8 = mybir.dt.float8e4
    bf16 = mybir.dt.bfloat16
    f32 = mybir.dt.float32
    Exp = mybir.ActivationFunctionType.Exp
    DR = mybir.MatmulPerfMode.DoubleRow

    if bass_cls is None:
        bass_cls = bacc.Bacc
    nc = bass_cls("TRN2", target_bir_lowering=False, debug=False,
                  enable_asserts=False, **bass_kw)
    wts_d = nc.dram_tensor("wts", [128, KT * NPC], f8, kind="ExternalInput")
    rhs_d = nc.dram_tensor("rhs", [128, 8192], f8, kind="ExternalInput")
    out_d = nc.dram_tensor("out", [2 * NPC, 2 * M], bf16, kind="ExternalOutput")

    with tile.TileContext(nc) as tc:
        with (
            tc.tile_pool(name="const", bufs=1) as cpool,
            tc.tile_pool(name="outp", bufs=2) as opool,
            tc.tile_pool(name="ep", bufs=3) as epool,
            tc.tile_pool(name="scp", bufs=3) as scpool,
            tc.tile_pool(name="ps", bufs=2, space="PSUM") as ppool,
        ):
            wts_sb = cpool.tile([128, KT * NPC], f8)
            nc.sync.dma_start(out=wts_sb[:], in_=wts_d[:, :])
            rhs_sb = cpool.tile([128, 8192], f8)
            # chunked so jgroup 0's CR block lands first
            for ch in range(4):
                nc.sync.dma_start(out=rhs_sb[:, ch * 2048:(ch + 1) * 2048],
                                  in_=rhs_d[:, ch * 2048:(ch + 1) * 2048])

            out_view = out_d.ap().rearrange("(i t) c -> i t c", t=2)

            for ib in range(NIB):
                out_all = opool.tile([IB, 4 * M], bf16, tag="out_all")
                for g in range(NJG):
                    gg = ib * NJG + g
                    b, hh = g // 2, g % 2
                    band = wts_sb[32 * b:32 * b + 12, :]
                    wt = band.rearrange("p (k m) -> p k m", k=KT)[
                        :, :, ib * IB:(ib + 1) * IB]
                    # rhs free layout per (band, hh): planar k-tiles,
                    # [Ck0|Rk0|Ck1|Rk1|AXk0|AXk1]; kt plane stride 1024.
                    rbase = rhs_sb[32 * b:32 * b + 12, :]
                    # Split PSUM tiles: WAR deps of future matmuls bind only
                    # to the consumers of the matching half. CR first so the
                    # longest consumer chain (exp -> C route) starts early.
                    mcr = ppool.tile([IB, 1024], f32, tag="memo_cr")
                    max_ = ppool.tile([IB, 1024], f32, tag="memo_ax")

                    def mm(out, off):
                        rh = BAP(rbase.tensor,
                                 rbase.offset + hh * 4096 + off,
                                 [list(rbase.ap)[0], [1024, 2], [1, 512]])
                        nc.tensor.matmul(
                            out, wt, rh, start=True, stop=True, perf_mode=DR,
                            tile_position=(32 * b, 0))

                    mm(mcr[:, 512:1024], 512)   # r2 plane
                    mm(mcr[:, 0:512], 0)        # C plane
                    ebig = epool.tile([IB, JG], f32, tag="e")
                    nc.scalar.activation(ebig[:], mcr[:, 512:1024], Exp,
                                         scale=-0.5)
                    mm(max_[:, 0:512], 2048)    # (A,X) pairs
                    mm(max_[:, 512:1024], 2560)

                    # h0 = (A,X) pairs * e
                    out0 = out_all[:, g * 1024:(g + 1) * 1024].rearrange(
                        "p (j t) -> p j t", t=2)
                    in0 = max_[:].rearrange("p (j t) -> p j t", t=2)
                    eb = ebig[:].unsqueeze(2).broadcast_to([IB, JG, 2])
                    nc.vector.tensor_mul(out0, in0, eb)

                    # h1 strided views (t=0 even col, t=1 odd col)
                    h1 = out_all[:, 8192 + g * 1024:8192 + (g + 1) * 1024]
                    h1 = h1.rearrange("p (j t) -> p j t", t=2)
                    h1e = h1[:, :, 0:1].squeeze(2)
                    h1o = h1[:, :, 1:2].squeeze(2)
                    h0o = out0[:, :, 1:2].squeeze(2)

                    if _c_on_dve(gg):
                        nc.vector.tensor_mul(h1o, mcr[:, 0:512], ebig[:])
                    else:
                        sc = scpool.tile([IB, JG], bf16, tag="sc")
                        nc.scalar.copy(sc[:], mcr[:, 0:512])
                        nc.gpsimd.tensor_mul(h1o, sc[:], ebig[:])
                    nc.scalar.copy(h1e, h0o)

                    # last i-block: per-jgroup DMAs to shrink the tail
                    last_ib = ib == NIB - 1
                    if last_ib or hh == 1:
                        i0 = ib * IB
                        w = 1024 if last_ib else 2048
                        cb = g * 1024 if last_ib else (g - 1) * 1024
                        nc.sync.dma_start(
                            out=out_view[i0:i0 + IB, 0:1, cb:cb + w].squeeze(1),
                            in_=out_all[:, cb:cb + w])
                        nc.sync.dma_start(
                            out=out_view[i0:i0 + IB, 1:2, cb:cb + w].squeeze(1),
                            in_=out_all[:, 8192 + cb:8192 + cb + w])
    nc.finalize()
    return nc


def _run(X, Y, log_length_scale, trace=False):
    from concourse.bass_utils import run_bass_kernel_spmd

    wts, rhs = _prepare_inputs(X, Y, log_length_scale)
    if "nc" not in _cache:
        _cache["nc"] = _build_module()
    nc = _cache["nc"]
    in_maps = [
        {
            "wts": np.ascontiguousarray(
                wts[:, :, c * NPC:(c + 1) * NPC].reshape(128, KT * NPC)),
            "rhs": rhs,
        }
        for c in range(NCORES)
    ]
    res = run_bass_kernel_spmd(nc, in_maps, core_ids=list(range(NCORES)),
                               trace=trace)
    out = np.concatenate([r["out"].astype(np.float32) for r in res.results],
                         axis=0)
    return out.reshape(1, 2 * N, 2 * M), res


def kernel(X, Y, log_length_scale):
    out, _ = _run(np.asarray(X), np.asarray(Y), np.asarray(log_length_scale))
    return out


# revision 14
# speedup vs baseline: 1.7011x; 1.1512x over previous
"""Divergence-free RBF kernel Gram matrix on 8 Trainium2 NeuronCores.

Math: for d=2, with scaled coords x' = x*exp(-ll/2):
  dx = x0_i - y0_j, dy = x1_i - y1_j, r2 = dx^2 + dy^2, e = exp(-r2/2)
  K[2i+0, 2j+0] = e * (1 - dy^2)          (channel A)
  K[2i+0, 2j+1] = K[2i+1, 2j+0] = e*dx*dy (channel X)
  K[2i+1, 2j+1] = e * (1 - dx^2)          (channel C)

Each polynomial factor is low-rank in the basis {1, x0, x1, x0*x1, x0^2,
x1^2}: host precomputes X-side rows L and Y-side coefficient columns; the
device builds A/X/C/r2 with PE matmuls (bf16 hi/lo 3-term, K=18), exp on ACT,
and one multiply-by-e per output element. Output is written bf16 (rel-err
budget 2e-2; this lands ~1.5e-3), halving HBM write traffic - the dominant
cost in this memory-bound problem.

PE trick: K=18 uses only 18 of 128 PE rows. The four matmuls of each 512-j
group (r2, C, and two A/X-pair halves) are issued to four different 32-row
tile_positions, whose moving data lives on four different SBUF partition
bands - the PE executes same-FIFO matmuls to distinct row groups
CONCURRENTLY, so a jgroup's PE time is ~1 matmul, not 4.

Consumers per jgroup, PSUM tiles [A,X pairs | C | r2]: ACT does exp(r2), DVE
multiplies the (A,X) block by e (broadcast over the pair dim), the C-plane
multiply alternates DVE-direct vs ACT-copy+GpSimd-multiply (GpSimd cannot
read PSUM), and the duplicate X column of the odd output row is an ACT SBUF
copy.

Sharding: rows of X (n axis) split across 8 cores, 512 each -> each core
writes 1024 rows of the (8192, 8192) Gram matrix. No communication.
"""

import numpy as np
import ml_dtypes

N = 4096          # X rows
M = 4096          # Y rows
NCORES = 8
NPC = N // NCORES  # 512 X rows per core
IB = 128           # i-block = partition count
NIB = NPC // IB    # 4 i-blocks per core
JG = 512           # j's per jgroup
NJG = M // JG      # 8 jgroups (per i-block)
KST = 18           # stacked contraction rows (3 x 6 basis)

_cache = {}


# C-plane multiply on DVE for ~5/8 of jgroups, else ACT-copy + GpSimd
# multiply; DVE route (shorter chain) for the final groups to cut the tail.
def _c_on_dve(gg):
    return gg % 8 < 5 or gg >= 29


def _hi_lo(a, dt):
    hi = a.astype(dt)
    lo = (a - hi.astype(np.float64)).astype(dt)
    return hi, lo


def _prepare_inputs(X, Y, log_length_scale):
    bf = ml_dtypes.bfloat16
    s = float(np.exp(-0.5 * np.float64(np.asarray(log_length_scale).reshape(-1)[0])))
    xs = np.asarray(X, dtype=np.float64).reshape(N, 2) * s
    ys = np.asarray(Y, dtype=np.float64).reshape(M, 2) * s
    x0, x1 = xs[:, 0], xs[:, 1]
    y0, y1 = ys[:, 0], ys[:, 1]
    one_n, zero_m, one_m = np.ones(N), np.zeros(M), np.ones(M)

    L = np.stack([one_n, x0, x1, x0 * x1, x0 ** 2, x1 ** 2])
    Lh, Ll = _hi_lo(L, bf)
    L18 = np.concatenate([Lh, Ll, Lh], axis=0)  # pairs with [Rh;Rh;Rl]

    c_00 = np.stack([1 - y1 ** 2, zero_m, 2 * y1, zero_m, zero_m, -one_m])
    c_dxdy = np.stack([y0 * y1, -y1, -y0, one_m, zero_m, zero_m])
    c_11 = np.stack([1 - y0 ** 2, 2 * y0, zero_m, zero_m, -one_m, zero_m])
    c_r2 = np.stack([y0 ** 2 + y1 ** 2, -2 * y0, -2 * y1, zero_m, one_m, one_m])

    def r18(c):
        ch, cl = _hi_lo(c, bf)
        return np.concatenate([ch, ch, cl], axis=0).astype(np.float64)

    R_A, R_X, R_C, R_R = r18(c_00), r18(c_dxdy), r18(c_11), r18(c_r2)

    # Stationary, replicated into the four 32-partition bands.
    wts = np.zeros((128, N), bf)
    for b in range(4):
        wts[32 * b:32 * b + KST, :] = L18

    # Moving tensor, one channel per band so each jgroup's 4 matmuls hit 4
    # different PE row groups (concurrent execution):
    #   band 0: C plane, band 1: r2 plane,
    #   band 2: (A,X) pairs j0..255 of each group, band 3: pairs j256..511.
    jidx = np.arange(M).reshape(NJG, JG)
    ax = np.zeros((KST, NJG, 2 * JG))
    ax[:, :, 0::2] = R_A[:, jidx]
    ax[:, :, 1::2] = R_X[:, jidx]
    rhs = np.zeros((128, 4096), bf)
    rhs[0:KST, :] = R_C[:, jidx].reshape(KST, 4096)
    rhs[32:32 + KST, :] = R_R[:, jidx].reshape(KST, 4096)
    rhs[64:64 + KST, :] = ax[:, :, 0:512].reshape(KST, 4096)
    rhs[96:96 + KST, :] = ax[:, :, 512:1024].reshape(KST, 4096)

    return wts, np.ascontiguousarray(rhs)


def _build_module(bass_cls=None, **bass_kw):
    from concourse import bacc, mybir
    import concourse.tile as tile

    bf16 = mybir.dt.bfloat16
    f32 = mybir.dt.float32
    Exp = mybir.ActivationFunctionType.Exp

    if bass_cls is None:
        bass_cls = bacc.Bacc
    nc = bass_cls("TRN2", target_bir_lowering=False, debug=False,
                  enable_asserts=False, **bass_kw)
    wts_d = nc.dram_tensor("wts", [128, NPC], bf16, kind="ExternalInput")
    rhs_d = nc.dram_tensor("rhs", [128, 4096], bf16, kind="ExternalInput")
    out_d = nc.dram_tensor("out", [2 * NPC, 2 * M], bf16, kind="ExternalOutput")

    with tile.TileContext(nc) as tc:
        with (
            tc.tile_pool(name="const", bufs=1) as cpool,
            tc.tile_pool(name="outp", bufs=2) as opool,
            tc.tile_pool(name="ep", bufs=3) as epool,
            tc.tile_pool(name="scp", bufs=3) as scpool,
            tc.tile_pool(name="ps", bufs=2, space="PSUM") as ppool,
        ):
            wts_sb = cpool.tile([128, NPC], bf16)
            rhs_sb = cpool.tile([128, 4096], bf16)
            # parallel-issue input loads from different engine queues;
            # chunked by jgroup pair so jgroup 0 can start ~1us in
            nc.scalar.dma_start(out=wts_sb[:], in_=wts_d[:, :])
            nc.sync.dma_start(out=rhs_sb[:, 0:1024], in_=rhs_d[:, 0:1024])
            nc.gpsimd.dma_start(out=rhs_sb[:, 1024:2048],
                                in_=rhs_d[:, 1024:2048])
            nc.sync.dma_start(out=rhs_sb[:, 2048:3072], in_=rhs_d[:, 2048:3072])
            nc.sync.dma_start(out=rhs_sb[:, 3072:4096], in_=rhs_d[:, 3072:4096])

            out_view = out_d.ap().rearrange("(i t) c -> i t c", t=2)

            for ib in range(NIB):
                out_all = opool.tile([IB, 4 * M], bf16, tag="out_all")
                for g in range(NJG):
                    gg = ib * NJG + g

                    def mm(out, band):
                        wt = wts_sb[32 * band:32 * band + KST,
                                    ib * IB:(ib + 1) * IB]
                        rh = rhs_sb[32 * band:32 * band + KST,
                                    g * JG:(g + 1) * JG]
                        nc.tensor.matmul(out, wt, rh, start=True, stop=True,
                                         tile_position=(32 * band, 0))

                    mcr = ppool.tile([IB, 1024], f32, tag="memo_cr")
                    max_ = ppool.tile([IB, 1024], f32, tag="memo_ax")
                    mm(mcr[:, 512:1024], 1)   # r2 plane
                    mm(mcr[:, 0:512], 0)      # C plane
                    ebig = epool.tile([IB, JG], f32, tag="e")
                    nc.scalar.activation(ebig[:], mcr[:, 512:1024], Exp,
                                         scale=-0.5)
                    mm(max_[:, 0:512], 2)     # (A,X) pairs, first half
                    mm(max_[:, 512:1024], 3)  # second half

                    # h0 = (A,X) pairs * e
                    out0 = out_all[:, g * 1024:(g + 1) * 1024].rearrange(
                        "p (j t) -> p j t", t=2)
                    in0 = max_[:].rearrange("p (j t) -> p j t", t=2)
                    eb = ebig[:].unsqueeze(2).broadcast_to([IB, JG, 2])
                    nc.vector.tensor_mul(out0, in0, eb)

                    # h1 strided views (t=0 even col, t=1 odd col)
                    h1 = out_all[:, 8192 + g * 1024:8192 + (g + 1) * 1024]
                    h1 = h1.rearrange("p (j t) -> p j t", t=2)
                    h1e = h1[:, :, 0:1].squeeze(2)
                    h1o = h1[:, :, 1:2].squeeze(2)
                    h0o = out0[:, :, 1:2].squeeze(2)

                    if _c_on_dve(gg):
                        nc.vector.tensor_mul(h1o, mcr[:, 0:512], ebig[:])
                    else:
                        sc = scpool.tile([IB, JG], bf16, tag="sc")
                        nc.scalar.copy(sc[:], mcr[:, 0:512])
                        nc.gpsimd.tensor_mul(h1o, sc[:], ebig[:])
                    nc.scalar.copy(h1e, h0o)

                    # last i-block: per-jgroup DMAs to shrink the tail
                    last_ib = ib == NIB - 1
                    if last_ib or g % 2 == 1:
                        i0 = ib * IB
                        w = 1024 if last_ib else 2048
                        cb = g * 1024 if last_ib else (g - 1) * 1024
                        nc.sync.dma_start(
                            out=out_view[i0:i0 + IB, 0:1, cb:cb + w].squeeze(1),
                            in_=out_all[:, cb:cb + w])
                        nc.sync.dma_start(
                            out=out_view[i0:i0 + IB, 1:2, cb:cb + w].squeeze(1),
                            in_=out_all[:, 8192 + cb:8192 + cb + w])
    nc.finalize()
    return nc


def _run(X, Y, log_length_scale, trace=False):
    from concourse.bass_utils import run_bass_kernel_spmd

    wts, rhs = _prepare_inputs(X, Y, log_length_scale)
    if "nc" not in _cache:
        _cache["nc"] = _build_module()
    nc = _cache["nc"]
    in_maps = [
        {
            "wts": np.ascontiguousarray(wts[:, c * NPC:(c + 1) * NPC]),
            "rhs": rhs,
        }
        for c in range(NCORES)
    ]
    res = run_bass_kernel_spmd(nc, in_maps, core_ids=list(range(NCORES)),
                               trace=trace)
    out = np.concatenate([r["out"].astype(np.float32) for r in res.results],
                         axis=0)
    return out.reshape(1, 2 * N, 2 * M), res


def kernel(X, Y, log_length_scale):
    out, _ = _run(np.asarray(X), np.asarray(Y), np.asarray(log_length_scale))
    return out


# revision 19
# speedup vs baseline: 1.7298x; 1.0169x over previous
"""Divergence-free RBF kernel Gram matrix on 8 Trainium2 NeuronCores.

Math: for d=2, with scaled coords x' = x*exp(-ll/2):
  dx = x0_i - y0_j, dy = x1_i - y1_j, r2 = dx^2 + dy^2, e = exp(-r2/2)
  K[2i+0, 2j+0] = e * (1 - dy^2)          (channel A)
  K[2i+0, 2j+1] = K[2i+1, 2j+0] = e*dx*dy (channel X)
  K[2i+1, 2j+1] = e * (1 - dx^2)          (channel C)

Each polynomial factor is low-rank in the basis {1, x0, x1, x0*x1, x0^2,
x1^2}: host precomputes X-side rows L and Y-side coefficient columns; the
device builds A/X/C/r2 with PE matmuls (bf16 hi/lo 3-term, K=18), exp on ACT,
and one multiply-by-e per output element. Output is written bf16 (rel-err
budget 2e-2; this lands ~1.5e-3), halving HBM write traffic - the dominant
cost in this memory-bound problem.

PE trick: K=18 uses only 18 of 128 PE rows. The four matmuls of each 512-j
group (r2, C, and two A/X-pair halves) are issued to four different 32-row
tile_positions, whose moving data lives on four different SBUF partition
bands - the PE executes same-FIFO matmuls to distinct row groups
CONCURRENTLY, so a jgroup's PE time is ~1 matmul, not 4.

Consumers per jgroup, PSUM tiles [A,X pairs | C | r2]: ACT does exp(r2), DVE
multiplies the (A,X) block by e (broadcast over the pair dim), the C-plane
multiply alternates DVE-direct vs ACT-copy+GpSimd-multiply (GpSimd cannot
read PSUM), and the duplicate X column of the odd output row is an ACT SBUF
copy.

Sharding: rows of X (n axis) split across 8 cores, 512 each -> each core
writes 1024 rows of the (8192, 8192) Gram matrix. No communication.
"""

import numpy as np
import ml_dtypes

N = 4096          # X rows
M = 4096          # Y rows
NCORES = 8
NPC = N // NCORES  # 512 X rows per core
IB = 128           # i-block = partition count
NIB = NPC // IB    # 4 i-blocks per core
JG = 512           # j's per jgroup
NJG = M // JG      # 8 jgroups (per i-block)
KST = 18           # stacked contraction rows (3 x 6 basis)

_cache = {}


# C-plane multiply on DVE for ~5/8 of jgroups, else ACT-copy + GpSimd
# multiply; DVE route (shorter chain) for the final groups to cut the tail.
def _c_on_dve(gg):
    return gg % 8 < 5 or gg >= 29


def _hi_lo(a, dt):
    hi = a.astype(dt)
    lo = (a - hi.astype(np.float64)).astype(dt)
    return hi, lo


def _prepare_inputs(X, Y, log_length_scale):
    bf = ml_dtypes.bfloat16
    s = float(np.exp(-0.5 * np.float64(np.asarray(log_length_scale).reshape(-1)[0])))
    xs = np.asarray(X, dtype=np.float64).reshape(N, 2) * s
    ys = np.asarray(Y, dtype=np.float64).reshape(M, 2) * s
    x0, x1 = xs[:, 0], xs[:, 1]
    y0, y1 = ys[:, 0], ys[:, 1]
    one_n, zero_m, one_m = np.ones(N), np.zeros(M), np.ones(M)

    L = np.stack([one_n, x0, x1, x0 * x1, x0 ** 2, x1 ** 2])
    Lh, Ll = _hi_lo(L, bf)
    L18 = np.concatenate([Lh, Ll, Lh], axis=0)  # pairs with [Rh;Rh;Rl]

    c_00 = np.stack([1 - y1 ** 2, zero_m, 2 * y1, zero_m, zero_m, -one_m])
    c_dxdy = np.stack([y0 * y1, -y1, -y0, one_m, zero_m, zero_m])
    c_11 = np.stack([1 - y0 ** 2, 2 * y0, zero_m, zero_m, -one_m, zero_m])
    c_r2 = np.stack([y0 ** 2 + y1 ** 2, -2 * y0, -2 * y1, zero_m, one_m, one_m])

    def r18(c):
        ch, cl = _hi_lo(c, bf)
        return np.concatenate([ch, ch, cl], axis=0).astype(np.float64)

    R_A, R_X, R_C, R_R = r18(c_00), r18(c_dxdy), r18(c_11), r18(c_r2)

    # Stationary, replicated into the four 32-partition bands.
    wts = np.zeros((128, N), bf)
    for b in range(4):
        wts[32 * b:32 * b + KST, :] = L18

    # Moving tensor, one channel per band so each jgroup's 4 matmuls hit 4
    # different PE row groups (concurrent execution):
    #   band 0: C plane, band 1: r2 plane,
    #   band 2: (A,X) pairs j0..255 of each group, band 3: pairs j256..511.
    jidx = np.arange(M).reshape(NJG, JG)
    ax = np.zeros((KST, NJG, 2 * JG))
    ax[:, :, 0::2] = R_A[:, jidx]
    ax[:, :, 1::2] = R_X[:, jidx]
    rhs = np.zeros((128, 4096), bf)
    rhs[0:KST, :] = R_C[:, jidx].reshape(KST, 4096)
    rhs[32:32 + KST, :] = R_R[:, jidx].reshape(KST, 4096)
    rhs[64:64 + KST, :] = ax[:, :, 0:512].reshape(KST, 4096)
    rhs[96:96 + KST, :] = ax[:, :, 512:1024].reshape(KST, 4096)

    return wts, np.ascontiguousarray(rhs)


def _build_module(bass_cls=None, **bass_kw):
    from concourse import bacc, mybir
    import concourse.tile as tile

    bf16 = mybir.dt.bfloat16
    f32 = mybir.dt.float32
    u32 = mybir.dt.uint32
    Exp = mybir.ActivationFunctionType.Exp

    if bass_cls is None:
        bass_cls = bacc.Bacc
    nc = bass_cls("TRN2", target_bir_lowering=False, debug=False,
                  enable_asserts=False, **bass_kw)
    wts_d = nc.dram_tensor("wts", [128, NPC], bf16, kind="ExternalInput")
    rhs_d = nc.dram_tensor("rhs", [128, 4096], bf16, kind="ExternalInput")
    out_d = nc.dram_tensor("out", [2 * NPC, 2 * M], bf16, kind="ExternalOutput")

    with tile.TileContext(nc) as tc:
        with (
            tc.tile_pool(name="const", bufs=1) as cpool,
            tc.tile_pool(name="outp", bufs=2) as opool,
            tc.tile_pool(name="ep", bufs=3) as epool,
            tc.tile_pool(name="scp", bufs=3) as scpool,
            tc.tile_pool(name="ps", bufs=2, space="PSUM") as ppool,
        ):
            wts_sb = cpool.tile([128, NPC], bf16)
            rhs_sb = cpool.tile([128, 4096], bf16)
            # parallel-issue input loads from different engine queues;
            # chunked by jgroup pair so jgroup 0 can start ~1us in
            nc.scalar.dma_start(out=wts_sb[:], in_=wts_d[:, :])
            nc.sync.dma_start(out=rhs_sb[:, 0:512], in_=rhs_d[:, 0:512])
            nc.sync.dma_start(out=rhs_sb[:, 512:1024], in_=rhs_d[:, 512:1024])
            nc.gpsimd.dma_start(out=rhs_sb[:, 1024:2048],
                                in_=rhs_d[:, 1024:2048])
            nc.sync.dma_start(out=rhs_sb[:, 2048:3072], in_=rhs_d[:, 2048:3072])
            nc.sync.dma_start(out=rhs_sb[:, 3072:4096], in_=rhs_d[:, 3072:4096])

            out_view = out_d.ap().rearrange("(i t) c -> i t c", t=2)

            for ib in range(NIB):
                out_all = opool.tile([IB, 4 * M], bf16, tag="out_all")
                i0 = ib * IB
                last_ib = ib == NIB - 1

                # X-duplicate, emitted one jgroup late (software pipeline):
                # it reads the pair-mul's output, so putting it at the head
                # of the ACT queue right away would block exp(g+1) behind a
                # DVE dependency.
                def x_copy(g):
                    out0 = out_all[:, g * 1024:(g + 1) * 1024].rearrange(
                        "p (j t) -> p j t", t=2)
                    h1 = out_all[:, 8192 + g * 1024:8192 + (g + 1) * 1024]
                    h1 = h1.rearrange("p (j t) -> p j t", t=2)
                    nc.scalar.copy(h1[:, :, 0:1].squeeze(2),
                                   out0[:, :, 1:2].squeeze(2))

                def flush_dma(g):
                    # h0/h1 rows of jgroup pair (g-1, g); h1 DMA waits on the
                    # x_copy and C-multiply of both groups
                    w = 1024 if last_ib else 2048
                    cb = g * 1024 if last_ib else (g - 1) * 1024
                    nc.scalar.dma_start(
                        out=out_view[i0:i0 + IB, 0:1, cb:cb + w].squeeze(1),
                        in_=out_all[:, cb:cb + w])
                    nc.sync.dma_start(
                        out=out_view[i0:i0 + IB, 1:2, cb:cb + w].squeeze(1),
                        in_=out_all[:, 8192 + cb:8192 + cb + w])

                for g in range(NJG):
                    gg = ib * NJG + g

                    def mm(out, band):
                        wt = wts_sb[32 * band:32 * band + KST,
                                    ib * IB:(ib + 1) * IB]
                        rh = rhs_sb[32 * band:32 * band + KST,
                                    g * JG:(g + 1) * JG]
                        nc.tensor.matmul(out, wt, rh, start=True, stop=True,
                                         tile_position=(32 * band, 0))

                    mcr = ppool.tile([IB, 1024], f32, tag="memo_cr")
                    max_ = ppool.tile([IB, 1024], f32, tag="memo_ax")
                    mm(mcr[:, 512:1024], 1)   # r2 plane
                    mm(mcr[:, 0:512], 0)      # C plane
                    ebig = epool.tile([IB, JG], f32, tag="e")
                    nc.scalar.activation(ebig[:], mcr[:, 512:1024], Exp,
                                         scale=-0.5)
                    mm(max_[:, 0:512], 2)     # (A,X) pairs, first half
                    mm(max_[:, 512:1024], 3)  # second half

                    # h0 = (A,X) pairs * e
                    out0 = out_all[:, g * 1024:(g + 1) * 1024].rearrange(
                        "p (j t) -> p j t", t=2)
                    in0 = max_[:].rearrange("p (j t) -> p j t", t=2)
                    eb = ebig[:].unsqueeze(2).broadcast_to([IB, JG, 2])
                    nc.vector.tensor_mul(out0, in0, eb)

                    # h1 odd cols = C * e
                    h1 = out_all[:, 8192 + g * 1024:8192 + (g + 1) * 1024]
                    h1 = h1.rearrange("p (j t) -> p j t", t=2)
                    h1o = h1[:, :, 1:2].squeeze(2)
                    if _c_on_dve(gg):
                        nc.vector.tensor_mul(h1o, mcr[:, 0:512], ebig[:])
                    else:
                        sc = scpool.tile([IB, JG], bf16, tag="sc")
                        nc.scalar.copy(sc[:], mcr[:, 0:512])
                        nc.gpsimd.tensor_mul(h1o, sc[:], ebig[:])

                    if g > 0:
                        x_copy(g - 1)
                        if last_ib:
                            flush_dma(g - 1)
                        elif g % 2 == 0:
                            flush_dma(g - 1)
                x_copy(NJG - 1)
                flush_dma(NJG - 1)
    nc.finalize()
    return nc


def _run(X, Y, log_length_scale, trace=False):
    from concourse.bass_utils import run_bass_kernel_spmd

    wts, rhs = _prepare_inputs(X, Y, log_length_scale)
    if "nc" not in _cache:
        _cache["nc"] = _build_module()
    nc = _cache["nc"]
    in_maps = [
        {
            "wts": np.ascontiguousarray(wts[:, c * NPC:(c + 1) * NPC]),
            "rhs": rhs,
        }
        for c in range(NCORES)
    ]
    res = run_bass_kernel_spmd(nc, in_maps, core_ids=list(range(NCORES)),
                               trace=trace)
    out = np.concatenate([r["out"].astype(np.float32) for r in res.results],
                         axis=0)
    return out.reshape(1, 2 * N, 2 * M), res


def kernel(X, Y, log_length_scale):
    out, _ = _run(np.asarray(X), np.asarray(Y), np.asarray(log_length_scale))
    return out
